# revision 12
# baseline (speedup 1.0000x reference)
"""ChannelAttention (B,D,H,W,C = 4,8,32,32,512; 8 heads, head_dim 64) on 8
Trainium2 NeuronCores, Bass/Tile SPMD. Fully data-parallel, zero cross-core
communication; cores (2j, 2j+1) split batch j's 8192 tokens in half.

Algebraic restructuring vs the straightforward schedule: channel attention's
per-head Gram matrix is a bilinear form of the input Gram,
    G_h = (Wk_h x^T) (x Wv_h^T) = Wk_h (x^T x) Wv_h^T,
so one S = x^T x (contract 8192 tokens, 512x512 out) replaces the k|v
projection of the whole batch; and because attn is only 64x64 per head, the
entire q -> out -> proj chain folds into a single fused 512x512 matrix
    M = sum_h Wq_h^T attn_h^T WpT_h,  y = x @ M + b,
so the per-token epilogue is one matmul instead of three.

All DRAM streams are host pre-permuted to partition-major [128, ...]
layouts (4-8 KiB contiguous per partition line). The x streams own the SP
hardware DMA queue; weights and the y output ride the Activation queue so
input never stalls behind output.

Schedule per core:
  S     : stream token-major x (bf16), accumulate upper-triangle blocks of
          S = x^T x into PSUM (N = 512/384/256/128 moving), HAM keeper
          matmuls warm the PE during the DMA fill. The 6 lower blocks are
          mirrored by PE transposes interleaved with the AT matmuls so the
          PE never waits on the PSUM->SBUF copies.
  chain : AT = S @ WkT (scale folded), G per head-pair via 256-wide f32r
          windows, softmax with fused bias+accum exp (table pre-warmed),
          P = attn^T @ WpT (block-diag pair packing, no transpose needed),
          M = Wq^T @ P contracted j-outer so each sweep starts as soon as
          its P block is copied.
  fused : stream own-half x^T (bf16), y = x @ M + bias, batched DMA out.

Numerics: S and the fused pass take bf16 inputs (fp32 PSUM accumulation),
the small chain matmuls are f32r. End-to-end L2 relative error vs the fp32
reference: ~4e-3.
"""

import numpy as np
import ml_dtypes
from contextlib import ExitStack

import concourse.bass as bass
import concourse.mybir as mybir
import concourse.tile as tile
from concourse import bacc
from concourse.bass_utils import run_bass_kernel_spmd
from concourse.masks import make_identity

B, D, H, W, C = 4, 8, 32, 32, 512
NUM_HEADS = 8
HEAD_DIM = C // NUM_HEADS
SCALE = HEAD_DIM ** -0.5
N_TOK = B * D * H * W
N_CORES = 8
N_LOC = N_TOK // N_CORES        # 4096 output tokens per core
N_BTOK = 2 * N_LOC              # 8192 tokens per batch (S contraction)
N_STILE = N_BTOK // 128         # 64 token tiles for S
CHUNK = 512                     # fused-pass chunk (tokens)
N_CHUNKS = N_LOC // CHUNK
N_CI = C // 128                 # 4 channel blocks
N_PAIRS = NUM_HEADS // 2

# x DMA chunking: two small lead chunks so the PE starts sooner
S_CHUNKS = [(0, 4), (4, 4)] + [(a, 8) for a in range(8, N_STILE, 8)]

f32 = mybir.dt.float32
f32r = mybir.dt.float32r
bf16 = mybir.dt.bfloat16

N_KEEP_START = 16
N_KEEP_MID = 8

_NC_CACHE = None


def build_nc():
    nc = bacc.Bacc(num_devices=N_CORES)

    xk = nc.declare_dram_parameter("xk", [128, N_STILE, C], bf16, isOutput=False)
    xT = nc.declare_dram_parameter("xT", [128, N_CHUNKS, N_CI, CHUNK], bf16, isOutput=False)
    wkT = nc.declare_dram_parameter("wkT", [128, N_CI, C], f32r, isOutput=False)
    wvT = nc.declare_dram_parameter("wvT", [128, N_CI, C], f32r, isOutput=False)
    wqh = nc.declare_dram_parameter("wqh", [128, N_CI, C], f32r, isOutput=False)
    wpT = nc.declare_dram_parameter("wpT", [128, N_CI, C], f32r, isOutput=False)
    bp = nc.declare_dram_parameter("bp", [1, C], f32r, isOutput=False)
    y = nc.declare_dram_parameter("y", [128, N_CHUNKS, N_CI, C], f32, isOutput=True)

    with tile.TileContext(nc) as tc, ExitStack() as ctx:
        const = ctx.enter_context(tc.tile_pool(name="const", bufs=1))
        persist = ctx.enter_context(tc.tile_pool(name="persist", bufs=1))
        sb = ctx.enter_context(tc.tile_pool(name="sb", bufs=2))

        # all weight loads ride the Activation HWDGE queue; SP carries x
        wkT_sb = const.tile([128, N_CI, C], f32r)
        nc.scalar.dma_start(wkT_sb[:], wkT[:])
        wvT_sb = const.tile([128, N_CI, C], f32r)
        nc.scalar.dma_start(wvT_sb[:], wvT[:])
        wqh_sb = const.tile([128, N_CI, C], f32r)
        nc.scalar.dma_start(wqh_sb[:], wqh[:])
        wpT_sb = const.tile([128, N_CI, C], f32r)
        nc.scalar.dma_start(wpT_sb[:], wpT[:])
        bp_f32 = const.tile([128, C], f32)
        bp_bcast = bass.AP(
            tensor=bp[:].bitcast(f32).tensor,
            offset=0,
            ap=[[0, 128], [1, C]],
        )
        nc.scalar.dma_start(bp_f32[:], bp_bcast)

        ones_f32 = const.tile([1, 128], f32)
        nc.vector.memset(ones_f32[:], 1.0)
        ones_sb = const.tile([1, 128], f32r)
        nc.vector.tensor_copy(ones_sb[:], ones_f32[:])
        zrow_f32 = const.tile([1, 512], f32)
        nc.vector.memset(zrow_f32[:], 0.0)
        zrow_sb = const.tile([1, 512], f32r)
        nc.vector.tensor_copy(zrow_sb[:], zrow_f32[:])
        ident = const.tile([128, 128], f32)
        make_identity(nc, ident[:])
        ident_r = const.tile([128, 128], f32r)
        nc.vector.tensor_copy(ident_r[:], ident[:])
        # pre-warm the scalar engine's Exp table off the critical chain
        warm = const.tile([1, 2], f32)
        nc.scalar.activation(warm[:], ones_f32[:, 0:2], mybir.ActivationFunctionType.Exp)
        # pre-zeroed block-diag pair lhsT (filled with probs later)
        atn_sb = persist.tile([128, N_PAIRS, 128], f32r)
        zro = const.tile([128, N_PAIRS, 128], f32)
        nc.vector.memset(zro[:], 0.0)
        nc.vector.tensor_copy(atn_sb[:], zro[:])

        S_sb = persist.tile([128, N_CI, C], f32r)
        red_sb = persist.tile([128, N_PAIRS, 64], f32)

        # ---------------- S = x^T x (upper triangle) -----------------------
        s_w = [C - 128 * i for i in range(N_CI)]  # 512, 384, 256, 128
        with tc.tile_pool(name="ps_s", bufs=1, space="PSUM") as ps_s:
            s_ps = [ps_s.tile([128, s_w[i]], f32, name=f"s{i}") for i in range(N_CI)]
            # bank-wide has_written seed + HAM warm-up during the DMA fill
            for i in range(N_KEEP_START):
                cb = i % N_CI
                nc.tensor.matmul(
                    s_ps[cb][:], ones_sb[:], zrow_sb[:, 0:s_w[cb]],
                    start=(i < N_CI), stop=False, skip_group_check=True,
                )

            for (a0, ntl) in S_CHUNKS:
                xt = sb.tile([128, ntl, C], bf16, tag="xtok", name=f"xt{a0}")
                nc.sync.dma_start(xt[:], xk[:, a0:a0 + ntl, :])
                for t in range(ntl):
                    last = (a0 + t == N_STILE - 1)
                    for cb in range(N_CI):
                        nc.tensor.matmul(
                            s_ps[cb][:],
                            xt[:, t, cb * 128:(cb + 1) * 128],
                            xt[:, t, cb * 128:C],
                            start=False, stop=last, skip_group_check=True,
                        )

            # upper blocks -> S_sb (f32r), split across vector/scalar engines
            for cb in range(N_CI):
                if cb % 2 == 0:
                    nc.vector.tensor_copy(S_sb[:, cb, cb * 128:C], s_ps[cb][:])
                else:
                    nc.scalar.copy(S_sb[:, cb, cb * 128:C], s_ps[cb][:])

        # Interleave: transposes that unblock AT column j run right
        # before AT's j sweep; the PSUM->SBUF copybacks (DVE) overlap
        # the next AT matmuls on the PE.
        with (
            tc.tile_pool(name="ps_tr", bufs=1, space="PSUM") as ps_tr,
            tc.tile_pool(name="ps_at", bufs=1, space="PSUM") as ps_at,
        ):
            tr_ps = ps_tr.tile([128, 6, 128], f32r)
            tr_of = {}
            idx = 0
            AT_ps = ps_at.tile([128, N_CI, C], f32)
            AT_sb = persist.tile([128, N_CI, C], f32r)

            def emit_tr(i, j):
                nonlocal idx
                nc.tensor.transpose(
                    tr_ps[:, idx, :], S_sb[:, j, i * 128:(i + 1) * 128], ident_r[:]
                )
                nc.vector.tensor_copy(S_sb[:, i, j * 128:(j + 1) * 128], tr_ps[:, idx, :])
                idx += 1

            for i in range(1, N_CI):
                emit_tr(i, 0)
            for j in range(N_CI):
                for i in range(j + 2, N_CI):
                    emit_tr(i, j + 1)
                for i in range(N_CI):
                    nc.tensor.matmul(
                        AT_ps[:, j, :],
                        S_sb[:, i, j * 128:(j + 1) * 128],
                        wkT_sb[:, i, :],
                        start=(i == 0), stop=(i == N_CI - 1),
                    )
                if j % 2 == 0:
                    nc.vector.tensor_copy(AT_sb[:, j, :], AT_ps[:, j, :])
                else:
                    nc.scalar.copy(AT_sb[:, j, :], AT_ps[:, j, :])

        # ---------------- G = AT-blocks @ WvT-blocks -----------------------
        with tc.tile_pool(name="ps_g", bufs=1, space="PSUM") as ps_g:
            # G per pair in a 256-wide window (f32r full rate needs N>=256);
            # j-outer so sweep j starts as soon as AT_sb[:, j] is copied.
            g_w = [min(p * 128, C - 256) for p in range(N_PAIRS)]
            G_ps = ps_g.tile([128, N_PAIRS, 256], f32)
            for j in range(N_CI):
                for p in range(N_PAIRS):
                    nc.tensor.matmul(
                        G_ps[:, p, :],
                        AT_sb[:, j, p * 128:(p + 1) * 128],
                        wvT_sb[:, j, g_w[p]:g_w[p] + 256],
                        start=(j == 0), stop=(j == N_CI - 1),
                        skip_group_check=True,
                    )

            # pack 8 useful 64x64 blocks -> red_sb[d + 64*(h%2), h//2, :]
            for h in range(NUM_HEADS):
                p = h // 2
                row0 = (h % 2) * 64
                col0 = p * 128 + row0 - g_w[p]
                src = G_ps[row0:row0 + 64, p, col0:col0 + 64]
                if h % 2 == 0:
                    nc.vector.tensor_copy(red_sb[row0:row0 + 64, p, :], src)
                else:
                    nc.scalar.copy(red_sb[row0:row0 + 64, p, :], src)

        with (
            tc.tile_pool(name="ps_p", bufs=1, space="PSUM") as ps_p,
            tc.tile_pool(name="ps_m", bufs=1, space="PSUM") as ps_m,
        ):
            P_ps = ps_p.tile([128, N_PAIRS, C], f32)
            # HAM keepers bridge the PE gap while softmax runs on DVE/ACT
            for i in range(N_KEEP_MID):
                nc.tensor.matmul(
                    P_ps[:, i % N_PAIRS, :], ones_sb[:], zrow_sb[:],
                    start=(i < N_PAIRS), stop=False, skip_group_check=True,
                )

            # ---- softmax over e on [128, pair, 64] ----
            nmax = sb.tile([128, N_PAIRS, 1], f32, tag="nmax")
            nc.vector.reduce_max(nmax[:], red_sb[:], axis=mybir.AxisListType.X, negate=True)
            shifted = sb.tile([128, N_PAIRS, 64], f32, tag="shifted")
            nc.vector.tensor_add(shifted[:], red_sb[:], nmax.broadcast_to([128, N_PAIRS, 64]))
            expd = sb.tile([128, N_PAIRS, 64], f32, tag="expd")
            nc.scalar.activation(expd[:], shifted[:], mybir.ActivationFunctionType.Exp)
            ssum = sb.tile([128, N_PAIRS, 1], f32, tag="ssum")
            nc.vector.reduce_sum(ssum[:], expd[:], axis=mybir.AxisListType.X)
            rsum = sb.tile([128, N_PAIRS, 1], f32, tag="rsum")
            nc.vector.reciprocal(rsum[:], ssum[:])
            probs = sb.tile([128, N_PAIRS, 64], f32, tag="probs")
            nc.vector.tensor_mul(probs[:], expd[:], rsum.broadcast_to([128, N_PAIRS, 64]))
            for p in range(N_PAIRS):
                nc.vector.tensor_copy(atn_sb[0:64, p, 0:64], probs[0:64, p, :])
                nc.vector.tensor_copy(atn_sb[64:128, p, 64:128], probs[64:128, p, :])

            # ---- P = attn^T @ WpT (pair block-diag) ; M = Wq^T @ P --------
            for p in range(N_PAIRS):
                nc.tensor.matmul(
                    P_ps[:, p, :], atn_sb[:, p, :], wpT_sb[:, p, :],
                    start=True, stop=True, skip_group_check=True,
                )
            P_sb = persist.tile([128, N_PAIRS, C], f32r)
            for p in range(N_PAIRS):
                if p % 2 == 0:
                    nc.vector.tensor_copy(P_sb[:, p, :], P_ps[:, p, :])
                else:
                    nc.scalar.copy(P_sb[:, p, :], P_ps[:, p, :])

            # M j-outer: sweep j starts as soon as P_sb[:, j] lands
            M_ps = ps_m.tile([128, N_CI, C], f32)
            for j in range(N_CI):
                for cb in range(N_CI):
                    nc.tensor.matmul(
                        M_ps[:, cb, :],
                        wqh_sb[:, j, cb * 128:(cb + 1) * 128],
                        P_sb[:, j, :],
                        start=(j == 0), stop=(j == N_CI - 1),
                        skip_group_check=True,
                    )
            M_sb = persist.tile([128, N_CI, C], bf16)
            for cb in range(N_CI):
                if cb % 2 == 0:
                    nc.vector.tensor_copy(M_sb[:, cb, :], M_ps[:, cb, :])
                else:
                    nc.scalar.copy(M_sb[:, cb, :], M_ps[:, cb, :])

        # ---------------- fused pass: y = x @ M + b ------------------------
        with tc.tile_pool(name="ps_y", bufs=3, space="PSUM") as ps_y:
            for c in range(N_CHUNKS):
                xt = sb.tile([128, N_CI, CHUNK], bf16, tag="xT", bufs=4)
                nc.sync.dma_start(xt[:], xT[:, c, :, :])
                y_acc = sb.tile([128, N_CI, C], f32, tag="yacc")
                for s in range(CHUNK // 128):
                    y_ps = ps_y.tile([128, C], f32, tag="y")
                    for cb in range(N_CI):
                        nc.tensor.matmul(
                            y_ps[:],
                            xt[:, cb, s * 128:(s + 1) * 128],
                            M_sb[:, cb, :],
                            start=(cb == 0), stop=(cb == N_CI - 1),
                        )
                    nc.vector.tensor_add(y_acc[:, s, :], y_ps[:], bp_f32[:])
                    if s % 2 == 1:
                        nc.scalar.dma_start(
                            y[:, c, s - 1:s + 1, :], y_acc[:, s - 1:s + 1, :]
                        )

    nc.compile()
    return nc


def _get_nc():
    global _NC_CACHE
    if _NC_CACHE is None:
        _NC_CACHE = build_nc()
    return _NC_CACHE


def prep_inputs(x, Wqkv, Wproj, bproj):
    x = np.ascontiguousarray(np.asarray(x, dtype=np.float32))
    Wqkv = np.asarray(Wqkv, dtype=np.float32)
    Wproj = np.asarray(Wproj, dtype=np.float32)
    bproj = np.asarray(bproj, dtype=np.float32)

    xf = x.reshape(B, N_BTOK, C)
    wq = Wqkv[0:C]                               # [he, c]
    wk_s = Wqkv[C:2 * C] * np.float32(SCALE)     # [kd, c], scale folded
    wv = Wqkv[2 * C:3 * C]                       # [vd, c]

    def pmaj(a):  # [512, 512] -> [128, 4, 512] partition-major
        return np.ascontiguousarray(a.reshape(N_CI, 128, C).transpose(1, 0, 2))

    wkT = pmaj(wk_s.T)
    wvT = pmaj(wv.T)
    wqh = pmaj(wq)
    wpT = pmaj(Wproj.T)
    bp = np.ascontiguousarray(bproj.reshape(1, C))

    # token-major x, pre-permuted: [128, 64 tiles, 512], token = a*128 + p
    xk_b = [
        np.ascontiguousarray(
            xf[b].reshape(N_STILE, 128, C).transpose(1, 0, 2)
        ).astype(ml_dtypes.bfloat16)
        for b in range(B)
    ]

    in_maps = []
    for i in range(N_CORES):
        b = i // 2
        t0 = (i % 2) * N_LOC
        # x^T pre-permuted: [128, chunk, cb, tok], c = cb*128 + p
        xTl = np.ascontiguousarray(
            xf[b, t0:t0 + N_LOC, :].T
            .reshape(N_CI, 128, N_CHUNKS, CHUNK).transpose(1, 2, 0, 3)
        ).astype(ml_dtypes.bfloat16)
        in_maps.append({
            "xk": xk_b[b], "xT": xTl, "wkT": wkT, "wvT": wvT,
            "wqh": wqh, "wpT": wpT, "bp": bp,
        })
    return in_maps


def gather_output(results):
    parts = []
    for i in range(N_CORES):
        yl = np.asarray(results[i]["y"])  # [128, chunk, s, 512]
        parts.append(yl.transpose(1, 2, 0, 3).reshape(N_LOC, C))
    return np.concatenate(parts, axis=0).reshape(B, D, H, W, C)


def kernel(x, Wqkv, Wproj, bproj, _trace=False, _tmpdir=None):
    nc = _get_nc()
    in_maps = prep_inputs(x, Wqkv, Wproj, bproj)
    res = run_bass_kernel_spmd(
        nc, in_maps, list(range(N_CORES)), trace=_trace, tmpdir=_tmpdir
    )
    out = gather_output(res.results)
    if _trace:
        kernel.last_exec_time_ns = res.exec_time_ns
        kernel.last_results = res
    return out


# revision 13
# speedup vs baseline: 1.0844x; 1.0844x over previous
"""ChannelAttention (B,D,H,W,C = 4,8,32,32,512; 8 heads, head_dim 64) on 8
Trainium2 NeuronCores, Bass/Tile SPMD. Fully data-parallel, zero cross-core
communication; cores (2j, 2j+1) split batch j's 8192 tokens in half.

Algebraic restructuring vs the straightforward schedule: channel attention's
per-head Gram matrix is a bilinear form of the input Gram,
    G_h = (Wk_h x^T) (x Wv_h^T) = Wk_h (x^T x) Wv_h^T,
so one S = x^T x (contract 8192 tokens, 512x512 out) replaces the k|v
projection of the whole batch; and because attn is only 64x64 per head, the
entire q -> out -> proj chain folds into a single fused 512x512 matrix
    M = sum_h Wq_h^T attn_h^T WpT_h,  y = x @ M + b,
so the per-token epilogue is one matmul instead of three.

All DRAM streams are host pre-permuted to partition-major [128, ...]
layouts (4-8 KiB contiguous per partition line). The x streams own the SP
hardware DMA queue; weights and the y output ride the Activation queue so
input never stalls behind output.

Schedule per core:
  S     : stream token-major x (bf16), accumulate upper-triangle blocks of
          S = x^T x into PSUM (N = 512/384/256/128 moving), HAM keeper
          matmuls warm the PE during the DMA fill. The 6 lower blocks are
          mirrored by PE transposes interleaved with the AT matmuls so the
          PE never waits on the PSUM->SBUF copies.
  chain : AT = S @ WkT (scale folded), G per head-pair via 256-wide f32r
          windows, softmax with fused bias+accum exp (table pre-warmed),
          P = attn^T @ WpT (block-diag pair packing, no transpose needed),
          M = Wq^T @ P contracted j-outer so each sweep starts as soon as
          its P block is copied.
  fused : stream own-half x^T (bf16), y = x @ M + bias, batched DMA out.

Numerics: S and the fused pass take bf16 inputs (fp32 PSUM accumulation),
the small chain matmuls are f32r. End-to-end L2 relative error vs the fp32
reference: ~4e-3.
"""

import numpy as np
import ml_dtypes
from contextlib import ExitStack

import concourse.bass as bass
import concourse.mybir as mybir
import concourse.tile as tile
from concourse import bacc
from concourse.bass_utils import run_bass_kernel_spmd
from concourse.masks import make_identity

B, D, H, W, C = 4, 8, 32, 32, 512
NUM_HEADS = 8
HEAD_DIM = C // NUM_HEADS
SCALE = HEAD_DIM ** -0.5
N_TOK = B * D * H * W
N_CORES = 8
N_LOC = N_TOK // N_CORES        # 4096 output tokens per core
N_BTOK = 2 * N_LOC              # 8192 tokens per batch (S contraction)
N_STILE = N_BTOK // 128         # 64 token tiles for S
CHUNK = 512                     # fused-pass chunk (tokens)
N_CHUNKS = N_LOC // CHUNK
N_CI = C // 128                 # 4 channel blocks
N_PAIRS = NUM_HEADS // 2

# x DMA chunking: two small lead chunks so the PE starts sooner
S_CHUNKS = [(0, 4), (4, 4)] + [(a, 8) for a in range(8, N_STILE, 8)]

f32 = mybir.dt.float32
f32r = mybir.dt.float32r
bf16 = mybir.dt.bfloat16

N_KEEP_START = 16
N_KEEP_MID = 8

_NC_CACHE = None


def build_nc():
    nc = bacc.Bacc(num_devices=N_CORES)

    xk = nc.declare_dram_parameter("xk", [128, N_STILE, C], bf16, isOutput=False)
    xT = nc.declare_dram_parameter("xT", [128, N_CHUNKS, N_CI, CHUNK], bf16, isOutput=False)
    wkT = nc.declare_dram_parameter("wkT", [128, N_CI, C], f32r, isOutput=False)
    wvT = nc.declare_dram_parameter("wvT", [128, N_CI, C], f32r, isOutput=False)
    wqh = nc.declare_dram_parameter("wqh", [128, N_CI, C], f32r, isOutput=False)
    wpT = nc.declare_dram_parameter("wpT", [128, N_CI, C], f32r, isOutput=False)
    bp = nc.declare_dram_parameter("bp", [1, C], f32r, isOutput=False)
    y = nc.declare_dram_parameter("y", [128, N_CHUNKS, N_CI, C], f32, isOutput=True)

    with tile.TileContext(nc) as tc, ExitStack() as ctx:
        const = ctx.enter_context(tc.tile_pool(name="const", bufs=1))
        persist = ctx.enter_context(tc.tile_pool(name="persist", bufs=1))
        sb = ctx.enter_context(tc.tile_pool(name="sb", bufs=2))

        # all weight loads ride the Activation HWDGE queue; SP carries x
        wkT_sb = const.tile([128, N_CI, C], f32r)
        nc.scalar.dma_start(wkT_sb[:], wkT[:])
        wvT_sb = const.tile([128, N_CI, C], f32r)
        nc.scalar.dma_start(wvT_sb[:], wvT[:])
        wqh_sb = const.tile([128, N_CI, C], f32r)
        nc.scalar.dma_start(wqh_sb[:], wqh[:])
        wpT_sb = const.tile([128, N_CI, C], f32r)
        nc.scalar.dma_start(wpT_sb[:], wpT[:])
        bp_f32 = const.tile([128, C], f32)
        bp_bcast = bass.AP(
            tensor=bp[:].bitcast(f32).tensor,
            offset=0,
            ap=[[0, 128], [1, C]],
        )
        nc.scalar.dma_start(bp_f32[:], bp_bcast)

        ones_f32 = const.tile([1, 128], f32)
        nc.vector.memset(ones_f32[:], 1.0)
        ones_sb = const.tile([1, 128], f32r)
        nc.vector.tensor_copy(ones_sb[:], ones_f32[:])
        zrow_f32 = const.tile([1, 512], f32)
        nc.vector.memset(zrow_f32[:], 0.0)
        zrow_sb = const.tile([1, 512], f32r)
        nc.vector.tensor_copy(zrow_sb[:], zrow_f32[:])
        ident = const.tile([128, 128], f32)
        make_identity(nc, ident[:])
        ident_r = const.tile([128, 128], f32r)
        nc.vector.tensor_copy(ident_r[:], ident[:])
        # pre-warm the scalar engine's Exp table off the critical chain
        warm = const.tile([1, 2], f32)
        nc.scalar.activation(warm[:], ones_f32[:, 0:2], mybir.ActivationFunctionType.Exp)
        # pre-zeroed block-diag pair lhsT (filled with probs later)
        atn_sb = persist.tile([128, N_PAIRS, 128], f32r)
        zro = const.tile([128, N_PAIRS, 128], f32)
        nc.vector.memset(zro[:], 0.0)
        nc.vector.tensor_copy(atn_sb[:], zro[:])

        S_sb = persist.tile([128, N_CI, C], f32r)
        red_sb = persist.tile([128, N_PAIRS, 64], f32)

        # ---------------- S = x^T x (upper triangle) -----------------------
        s_w = [C - 128 * i for i in range(N_CI)]  # 512, 384, 256, 128
        with tc.tile_pool(name="ps_s", bufs=1, space="PSUM") as ps_s:
            s_ps = [ps_s.tile([128, s_w[i]], f32, name=f"s{i}") for i in range(N_CI)]
            # bank-wide has_written seed + HAM warm-up during the DMA fill
            for i in range(N_KEEP_START):
                cb = i % N_CI
                nc.tensor.matmul(
                    s_ps[cb][:], ones_sb[:], zrow_sb[:, 0:s_w[cb]],
                    start=(i < N_CI), stop=False, skip_group_check=True,
                )

            for (a0, ntl) in S_CHUNKS:
                xt = sb.tile([128, ntl, C], bf16, tag="xtok", name=f"xt{a0}")
                nc.sync.dma_start(xt[:], xk[:, a0:a0 + ntl, :])
                for t in range(ntl):
                    last = (a0 + t == N_STILE - 1)
                    for cb in range(N_CI):
                        nc.tensor.matmul(
                            s_ps[cb][:],
                            xt[:, t, cb * 128:(cb + 1) * 128],
                            xt[:, t, cb * 128:C],
                            start=False, stop=last, skip_group_check=True,
                        )

            # upper blocks -> S_sb (f32r), split across vector/scalar engines
            for cb in range(N_CI):
                if cb % 2 == 0:
                    nc.vector.tensor_copy(S_sb[:, cb, cb * 128:C], s_ps[cb][:])
                else:
                    nc.scalar.copy(S_sb[:, cb, cb * 128:C], s_ps[cb][:])

        # Interleave: transposes that unblock AT column j run right
        # before AT's j sweep; the PSUM->SBUF copybacks (DVE) overlap
        # the next AT matmuls on the PE.
        with (
            tc.tile_pool(name="ps_tr", bufs=1, space="PSUM") as ps_tr,
            tc.tile_pool(name="ps_at", bufs=1, space="PSUM") as ps_at,
        ):
            tr_ps = ps_tr.tile([128, 6, 128], f32r)
            tr_of = {}
            idx = 0
            AT_ps = ps_at.tile([128, N_CI, C], f32)
            AT_sb = persist.tile([128, N_CI, C], f32r)

            def emit_tr(i, j):
                nonlocal idx
                nc.tensor.transpose(
                    tr_ps[:, idx, :], S_sb[:, j, i * 128:(i + 1) * 128], ident_r[:]
                )
                nc.vector.tensor_copy(S_sb[:, i, j * 128:(j + 1) * 128], tr_ps[:, idx, :])
                idx += 1

            for i in range(1, N_CI):
                emit_tr(i, 0)
            for j in range(N_CI):
                for i in range(j + 2, N_CI):
                    emit_tr(i, j + 1)
                for i in range(N_CI):
                    nc.tensor.matmul(
                        AT_ps[:, j, :],
                        S_sb[:, i, j * 128:(j + 1) * 128],
                        wkT_sb[:, i, :],
                        start=(i == 0), stop=(i == N_CI - 1),
                    )
                if j % 2 == 0:
                    nc.vector.tensor_copy(AT_sb[:, j, :], AT_ps[:, j, :])
                else:
                    nc.scalar.copy(AT_sb[:, j, :], AT_ps[:, j, :])

        # ---------------- G = AT-blocks @ WvT-blocks -----------------------
        with tc.tile_pool(name="ps_g", bufs=1, space="PSUM") as ps_g:
            # G per pair in a 256-wide window (f32r full rate needs N>=256);
            # j-outer so sweep j starts as soon as AT_sb[:, j] is copied.
            g_w = [min(p * 128, C - 256) for p in range(N_PAIRS)]
            G_ps = ps_g.tile([128, N_PAIRS, 256], f32)
            # bank-wide zero seed: a start=True on one pair's region would
            # invalidate the other pair's in-flight accumulation in the
            # same PSUM bank, so seed whole banks once and only accumulate
            for b in range(2):
                nc.tensor.matmul(
                    G_ps[:, 2 * b:2 * b + 2, :].rearrange("p a e -> p (a e)"),
                    ones_sb[:], zrow_sb[:],
                    start=True, stop=False, skip_group_check=True,
                )
            for j in range(N_CI):
                for p in range(N_PAIRS):
                    nc.tensor.matmul(
                        G_ps[:, p, :],
                        AT_sb[:, j, p * 128:(p + 1) * 128],
                        wvT_sb[:, j, g_w[p]:g_w[p] + 256],
                        start=False, stop=(j == N_CI - 1),
                        skip_group_check=True,
                    )

            # pack 8 useful 64x64 blocks -> red_sb[d + 64*(h%2), h//2, :]
            for h in range(NUM_HEADS):
                p = h // 2
                row0 = (h % 2) * 64
                col0 = p * 128 + row0 - g_w[p]
                src = G_ps[row0:row0 + 64, p, col0:col0 + 64]
                if h % 2 == 0:
                    nc.vector.tensor_copy(red_sb[row0:row0 + 64, p, :], src)
                else:
                    nc.scalar.copy(red_sb[row0:row0 + 64, p, :], src)

        with (
            tc.tile_pool(name="ps_p", bufs=1, space="PSUM") as ps_p,
            tc.tile_pool(name="ps_m", bufs=1, space="PSUM") as ps_m,
        ):
            P_ps = ps_p.tile([128, N_PAIRS, C], f32)
            # HAM keepers bridge the PE gap while softmax runs on DVE/ACT
            for i in range(N_KEEP_MID):
                nc.tensor.matmul(
                    P_ps[:, i % N_PAIRS, :], ones_sb[:], zrow_sb[:],
                    start=(i < N_PAIRS), stop=False, skip_group_check=True,
                )

            # ---- softmax over e on [128, pair, 64] ----
            nmax = sb.tile([128, N_PAIRS, 1], f32, tag="nmax")
            nc.vector.reduce_max(nmax[:], red_sb[:], axis=mybir.AxisListType.X, negate=True)
            shifted = sb.tile([128, N_PAIRS, 64], f32, tag="shifted")
            nc.vector.tensor_add(shifted[:], red_sb[:], nmax.broadcast_to([128, N_PAIRS, 64]))
            expd = sb.tile([128, N_PAIRS, 64], f32, tag="expd")
            nc.scalar.activation(expd[:], shifted[:], mybir.ActivationFunctionType.Exp)
            ssum = sb.tile([128, N_PAIRS, 1], f32, tag="ssum")
            nc.vector.reduce_sum(ssum[:], expd[:], axis=mybir.AxisListType.X)
            rsum = sb.tile([128, N_PAIRS, 1], f32, tag="rsum")
            nc.vector.reciprocal(rsum[:], ssum[:])
            probs = sb.tile([128, N_PAIRS, 64], f32, tag="probs")
            nc.vector.tensor_mul(probs[:], expd[:], rsum.broadcast_to([128, N_PAIRS, 64]))
            for p in range(N_PAIRS):
                nc.vector.tensor_copy(atn_sb[0:64, p, 0:64], probs[0:64, p, :])
                nc.vector.tensor_copy(atn_sb[64:128, p, 64:128], probs[64:128, p, :])

            # ---- P = attn^T @ WpT (pair block-diag) ; M = Wq^T @ P --------
            for p in range(N_PAIRS):
                nc.tensor.matmul(
                    P_ps[:, p, :], atn_sb[:, p, :], wpT_sb[:, p, :],
                    start=True, stop=True, skip_group_check=True,
                )
            P_sb = persist.tile([128, N_PAIRS, C], f32r)
            for p in range(N_PAIRS):
                if p % 2 == 0:
                    nc.vector.tensor_copy(P_sb[:, p, :], P_ps[:, p, :])
                else:
                    nc.scalar.copy(P_sb[:, p, :], P_ps[:, p, :])

            # M j-outer: sweep j starts as soon as P_sb[:, j] lands
            M_ps = ps_m.tile([128, N_CI, C], f32)
            for j in range(N_CI):
                for cb in range(N_CI):
                    nc.tensor.matmul(
                        M_ps[:, cb, :],
                        wqh_sb[:, j, cb * 128:(cb + 1) * 128],
                        P_sb[:, j, :],
                        start=(j == 0), stop=(j == N_CI - 1),
                        skip_group_check=True,
                    )
            M_sb = persist.tile([128, N_CI, C], bf16)
            for cb in range(N_CI):
                if cb % 2 == 0:
                    nc.vector.tensor_copy(M_sb[:, cb, :], M_ps[:, cb, :])
                else:
                    nc.scalar.copy(M_sb[:, cb, :], M_ps[:, cb, :])

        # ---------------- fused pass: y = x @ M + b ------------------------
        with tc.tile_pool(name="ps_y", bufs=3, space="PSUM") as ps_y:
            for c in range(N_CHUNKS):
                xt = sb.tile([128, N_CI, CHUNK], bf16, tag="xT", bufs=4)
                nc.sync.dma_start(xt[:], xT[:, c, :, :])
                y_acc = sb.tile([128, N_CI, C], f32, tag="yacc")
                for s in range(CHUNK // 128):
                    y_ps = ps_y.tile([128, C], f32, tag="y")
                    for cb in range(N_CI):
                        nc.tensor.matmul(
                            y_ps[:],
                            xt[:, cb, s * 128:(s + 1) * 128],
                            M_sb[:, cb, :],
                            start=(cb == 0), stop=(cb == N_CI - 1),
                        )
                    nc.vector.tensor_add(y_acc[:, s, :], y_ps[:], bp_f32[:])
                    if s % 2 == 1:
                        nc.scalar.dma_start(
                            y[:, c, s - 1:s + 1, :], y_acc[:, s - 1:s + 1, :]
                        )

    nc.compile()
    return nc


def _get_nc():
    global _NC_CACHE
    if _NC_CACHE is None:
        _NC_CACHE = build_nc()
    return _NC_CACHE


def prep_inputs(x, Wqkv, Wproj, bproj):
    x = np.ascontiguousarray(np.asarray(x, dtype=np.float32))
    Wqkv = np.asarray(Wqkv, dtype=np.float32)
    Wproj = np.asarray(Wproj, dtype=np.float32)
    bproj = np.asarray(bproj, dtype=np.float32)

    xf = x.reshape(B, N_BTOK, C)
    wq = Wqkv[0:C]                               # [he, c]
    wk_s = Wqkv[C:2 * C] * np.float32(SCALE)     # [kd, c], scale folded
    wv = Wqkv[2 * C:3 * C]                       # [vd, c]

    def pmaj(a):  # [512, 512] -> [128, 4, 512] partition-major
        return np.ascontiguousarray(a.reshape(N_CI, 128, C).transpose(1, 0, 2))

    wkT = pmaj(wk_s.T)
    wvT = pmaj(wv.T)
    wqh = pmaj(wq)
    wpT = pmaj(Wproj.T)
    bp = np.ascontiguousarray(bproj.reshape(1, C))

    # token-major x, pre-permuted: [128, 64 tiles, 512], token = a*128 + p
    xk_b = [
        np.ascontiguousarray(
            xf[b].reshape(N_STILE, 128, C).transpose(1, 0, 2)
        ).astype(ml_dtypes.bfloat16)
        for b in range(B)
    ]

    in_maps = []
    for i in range(N_CORES):
        b = i // 2
        t0 = (i % 2) * N_LOC
        # x^T pre-permuted: [128, chunk, cb, tok], c = cb*128 + p
        xTl = np.ascontiguousarray(
            xf[b, t0:t0 + N_LOC, :].T
            .reshape(N_CI, 128, N_CHUNKS, CHUNK).transpose(1, 2, 0, 3)
        ).astype(ml_dtypes.bfloat16)
        in_maps.append({
            "xk": xk_b[b], "xT": xTl, "wkT": wkT, "wvT": wvT,
            "wqh": wqh, "wpT": wpT, "bp": bp,
        })
    return in_maps


def gather_output(results):
    parts = []
    for i in range(N_CORES):
        yl = np.asarray(results[i]["y"])  # [128, chunk, s, 512]
        parts.append(yl.transpose(1, 2, 0, 3).reshape(N_LOC, C))
    return np.concatenate(parts, axis=0).reshape(B, D, H, W, C)


def kernel(x, Wqkv, Wproj, bproj, _trace=False, _tmpdir=None):
    nc = _get_nc()
    in_maps = prep_inputs(x, Wqkv, Wproj, bproj)
    res = run_bass_kernel_spmd(
        nc, in_maps, list(range(N_CORES)), trace=_trace, tmpdir=_tmpdir
    )
    out = gather_output(res.results)
    if _trace:
        kernel.last_exec_time_ns = res.exec_time_ns
        kernel.last_results = res
    return out
